# revision 12
# baseline (speedup 1.0000x reference)
"""Butterfly layer (12 stages over L=4096) on 8 Trainium2 NeuronCores.

Math: each stage s computes y[j] = W[s][j,0]*y[j] + W[s][j,1]*y[j^d], d=2^s.
The product of stages 0-6 is block-diagonal over contiguous 128-blocks of L
(32 dense 128x128 matrices C1).  Stages 7-11 mix positions {m*128+p : m} at
fixed p, i.e. a 32x32 dense matrix F_p per within-block position p; grouping
p by residue class mod 32 turns these into 32 block-diagonal 128x128
stationaries C2 after a 32x32 partition<->free exchange done on-chip by the
DVE stream transpose.

Numerics: C1 rows are L2-normalized (scale s1 folded into C2) so the
intermediate y1 is ~N(0,1) in fp16.  C2 columns additionally fold a per-row
output scale s2 = 7.5*sigma_row/127 (sigma_row = exact L2 row norm of the
full butterfly matrix, computed from W on the host), so the phase-2 PSUM
values are int8-ranged; the PSUM->SBUF drain casts straight to int8 (the
HW cast is round-to-nearest-even and saturating — probed on device; the
CoreSim model truncates/wraps and overstates the error) and the output
travels over HBM as int8 (4 MiB/core instead of 8 MiB fp16).  The host
multiplies rows by s2 to dequantize.  Measured on-device max rel err
0.835e-2 (scale-relative, tolerance 2e-2); int8 wrap margin 33%.  8-bit
input or intermediate formats fail: the butterfly row norms span 3+ orders
of magnitude, so a global-scale-relative budget needs >=10 effective input
bits.

Device layout is transposed ([L on partitions, batch on free]); the host
transposes x / untransposes the result.  Data-parallel over batch across
the 8 cores.  x travels as fp16 (8 MiB/core); per-iteration HBM traffic is
12 MiB/core vs 16 MiB for the fp16-out baseline.  The kernel is now
engine-bound, not DMA-bound: the two PSUM->SBUF drains (1 elem/cyc/lane,
f32 source) plus the DVE pairs-transpose total ~43 us/iter across ACT+DVE
(no_io ablation), ~47 us with IO overlap losses.  PSUM drain copies are
left to the Tile scheduler (nc.any) — measured ~4 us faster than a static
ACT/DVE split.  Output DMA triggers ride the idle gpsimd queue so they
are not delayed behind ACT's drain queue.
"""
import numpy as np

BATCH, L, E = 8192, 4096, 12
N_CORES = 8
BCORE = BATCH // N_CORES  # 1024 batch columns per core
BC = 256                  # batch columns per superchunk
NSC = BCORE // BC         # 4 superchunks per core
NB = L // 128             # 32 partition blocks
OUT_R = 7.5               # int8 output range in sigma units

_CACHE = {}


def _build_coeffs(W):
    """c1 (L,128) f16 row-normalized, c2 (L,128) f16 with s1-unfold and
    1/s2 output scaling folded in, s2_flat (L,) f64 host dequant scales."""
    Wd = np.asarray(W, dtype=np.float64)
    A, B = Wd[:, :, 0], Wd[:, :, 1]

    # stages 0..6 per 128-block
    Em_all = np.empty((NB, 128, 128))
    for m in range(NB):
        Em = np.eye(128)
        for s in range(7):
            d = 1 << s
            a = A[s, m * 128:(m + 1) * 128][:, None]
            b = B[s, m * 128:(m + 1) * 128][:, None]
            idx = np.arange(128) ^ d
            Em = a * Em + b * Em[idx, :]
        Em_all[m] = Em
    s1 = np.sqrt((Em_all ** 2).sum(axis=2))          # (m, mu) row norms
    Em_n = Em_all / s1[:, :, None]
    # c1[m*128+k, mu] = Em_n[m][mu, k]  (lhsT layout)
    c1 = Em_n.transpose(0, 2, 1).reshape(L, 128).astype(np.float16)

    # stages 7..11 per position p: F_p (32x32 over block index)
    idx32 = np.arange(32)
    F = np.empty((128, 32, 32))
    for p in range(128):
        Fp = np.eye(32)
        for s in range(7, 12):
            e = (1 << s) // 128
            a = A[s, idx32 * 128 + p][:, None]
            b = B[s, idx32 * 128 + p][:, None]
            Fp = a * Fp + b * Fp[idx32 ^ e, :]
        F[p] = Fp
    # sigma_out^2[m, p] = sum_m' F_p[m, m']^2 * s1[m', p]^2
    sigma_out = np.sqrt(np.einsum("pmn,np->mp", F ** 2, s1 ** 2))
    s2 = OUT_R * sigma_out / 127.0                   # (m, p)

    # c2[c*128 + 32q + m', 4m + q] = F_p[m, m'] * s1[m', p] / s2[m, p],
    # p = 32q + c.  Output DRAM row t*32+c with t=4m+q equals m*128+32q+c.
    c2 = np.zeros((L, 128))
    for c in range(32):
        for q in range(4):
            p = 32 * q + c
            blk = F[p] * s1[:, p][None, :] / s2[:, p][:, None]  # (m, m')
            # rows 32q+m', cols 4m+q
            rows = c * 128 + 32 * q + idx32          # m' axis
            cols = 4 * idx32 + q                     # m axis
            c2[np.ix_(rows, cols)] = blk.T           # [m', m]
    s2_flat = s2.reshape(L)                          # j = m*128 + p order
    return c1, c2.astype(np.float16), s2_flat


def _split_excess_waits(nc):
    """The staged walrus rejects instructions carrying more than one sync-wait
    command.  Hoist all but the last semaphore wait of each instruction onto
    fresh same-engine nops placed immediately before it (engines execute
    their stream in order, so semantics are unchanged)."""
    from concourse import mybir

    snapshots = []
    for fn in nc.m.functions:
        for blk in fn.blocks:
            snapshots.append((blk, list(blk.instructions)))
    for blk, insts in snapshots:
        changed = False
        new_list = []
        for inst in insts:
            si = inst.sync_info
            if si is not None:
                waits = list(si.on_wait)
                sem_waits = [w for w in waits if w.sync_type == "semaphore"]
                other = [w for w in waits if w.sync_type != "semaphore"]
                budget = 1 if not other else 0
                if len(sem_waits) > budget:
                    keep = sem_waits[-budget:] if budget else []
                    hoist = sem_waits[: len(sem_waits) - budget]
                    for w in hoist:
                        nop = nc.engines[inst.engine].nop(
                            hint="waitsplit", nofuse=True
                        )
                        nop.ins.sync_info = mybir.SyncInfo(
                            on_wait=[w], on_update=[]
                        )
                        new_list.append(nop.ins)
                    si.on_wait = other + keep
                    changed = True
            new_list.append(inst)
        if changed:
            blk.instructions = new_list


def _build_program(repeat: int = 1, no_io: bool = False, split: str = "any"):
    import concourse.bass as bass
    import concourse.tile as tile
    from concourse import mybir

    f32 = mybir.dt.float32
    f16 = mybir.dt.float16
    i8 = mybir.dt.int8

    nc = bass.Bass("TRN2", num_devices=N_CORES)
    xT = nc.dram_tensor("xT", [L, BCORE], f16, kind="ExternalInput").ap()
    c1 = nc.dram_tensor("c1", [L, 128], f16, kind="ExternalInput").ap()
    c2 = nc.dram_tensor("c2", [L, 128], f16, kind="ExternalInput").ap()
    outT = nc.dram_tensor("outT", [L, BCORE], i8, kind="ExternalOutput").ap()

    xT_r = xT.rearrange("(m p) b -> p m b", p=128)    # [128, 32, BCORE]
    c1_r = c1.rearrange("(m p) k -> p m k", p=128)    # [128, 32, 128]
    c2_r = c2.rearrange("(m p) k -> p m k", p=128)
    outT_r = outT.rearrange("(t c) b -> t c b", c=32)  # [128, 32, BCORE]

    with tile.TileContext(nc) as tc:
        with (
            tc.tile_pool(name="const", bufs=1) as cpool,
            tc.tile_pool(name="xin", bufs=2) as xpool,
            tc.tile_pool(name="y1", bufs=2) as y1pool,
            tc.tile_pool(name="y1t", bufs=2) as y1tpool,
            tc.tile_pool(name="yo", bufs=10) as yopool,
            tc.tile_pool(name="ps1", bufs=2, space="PSUM") as ps1pool,
            tc.tile_pool(name="ps2", bufs=2, space="PSUM") as ps2pool,
        ):
            c1t = cpool.tile([128, NB * 128], f16, tag="c1t")
            nc.sync.dma_start(
                c1t[:].rearrange("p (m k) -> p m k", m=NB), c1_r[:]
            )
            c2t = cpool.tile([128, NB * 128], f16, tag="c2t")
            nc.sync.dma_start(
                c2t[:].rearrange("p (m k) -> p m k", m=NB), c2_r[:]
            )

            xin_shared = None
            if no_io:
                xin_shared = xpool.tile([128, NB * 2 * BC], f16, tag="xin")
                for h in range(4):
                    nc.sync.dma_start(
                        xin_shared[
                            :, h * 8 * 2 * BC:(h + 1) * 8 * 2 * BC
                        ].rearrange("p (m b) -> p m b", m=8),
                        xT_r[:, h * 8:(h + 1) * 8, 0:2 * BC],
                    )

            yo_group = {}
            for s in range(NSC * repeat):
                s = s % NSC
                # two superchunks share one xin tile -> 1 KiB DMA runs
                if no_io:
                    xin2 = xin_shared
                elif s % 2 == 0:
                    xin2 = xpool.tile([128, NB * 2 * BC], f16, tag="xin")
                    for h in range(4):
                        nc.sync.dma_start(
                            xin2[
                                :, h * 8 * 2 * BC:(h + 1) * 8 * 2 * BC
                            ].rearrange("p (m b) -> p m b", m=8),
                            xT_r[
                                :, h * 8:(h + 1) * 8,
                                s * BC:(s + 2) * BC,
                            ],
                        )
                xoff = (s % 2) * BC
                xin3 = xin2[:].rearrange("p (m b) -> p m b", b=2 * BC)
                y1 = y1pool.tile([128, NB * BC], f16, tag="y1")
                y1t = y1tpool.tile([128, NB * BC], f16, tag="y1t")
                for g in range(8):
                    ps = ps1pool.tile([128, 4 * BC], f32, tag="ps1")
                    for i in range(4):
                        m = 4 * g + i
                        nc.tensor.matmul(
                            ps[:, i * BC:(i + 1) * BC],
                            c1t[:, m * 128:(m + 1) * 128],
                            xin3[:, m, xoff:xoff + BC],
                            start=True,
                            stop=True,
                        )
                    # explicit ACT/DVE split: ACT is 1.25x faster per drained
                    # element and DVE also carries the transposes
                    y1dst = y1[:].rearrange(
                        "p (k m t) -> p m k t", m=NB, t=2
                    )[:, 4 * g:4 * (g + 1), :, :]
                    pssrc = ps[:].rearrange("p (m k t) -> p m k t", m=4, t=2)
                    if split == "any":
                        nc.any.tensor_copy(y1dst, pssrc)
                    elif g < 6:
                        nc.scalar.copy(y1dst, pssrc)
                    else:
                        nc.vector.tensor_copy(y1dst, pssrc)
                # 32x32 partition<->free exchange on fp16 pairs (bitcast f32)
                nc.vector.transpose(
                    y1t[:].bitcast(mybir.dt.float32),
                    y1[:].bitcast(mybir.dt.float32),
                )
                shalf = s % 2
                for g in range(8):
                    # each yo tile accumulates two superchunks (512 B output
                    # DMA rows, the line-rate minimum) so output DMA spreads
                    # across the iteration instead of bunching at the end
                    if shalf == 0:
                        yo2 = yopool.tile([128, 4 * 2 * BC], i8, tag="yo")
                        yo_group[g] = yo2
                    else:
                        yo2 = yo_group[g]
                    ps2 = ps2pool.tile([128, 4 * BC], f32, tag="ps2")
                    for i in range(4):
                        c = 4 * g + i
                        nc.tensor.matmul(
                            ps2[:, i * BC:(i + 1) * BC],
                            c2t[:, c * 128:(c + 1) * 128],
                            y1t[:].rearrange(
                                "p (k c t) -> p c k t", c=NB, t=2
                            )[:, c, :, :],
                            start=True,
                            stop=True,
                        )
                    yodst = yo2[:].rearrange(
                        "p (c h b) -> p h c b", c=4, h=2
                    )[:, shalf, :, :]
                    ps2src = ps2[:].rearrange("p (c b) -> p c b", c=4)
                    if split == "any":
                        nc.any.tensor_copy(yodst, ps2src)
                    elif g < 5:
                        nc.scalar.copy(yodst, ps2src)
                    else:
                        nc.vector.tensor_copy(yodst, ps2src)
                    if shalf == 1 and not no_io:
                        # gpsimd queue: its trigger slots are idle (ACT/DVE
                        # carry the PSUM drains, SP carries the input DMA)
                        nc.gpsimd.dma_start(
                            outT_r[
                                :, 4 * g:4 * (g + 1),
                                (s - 1) * BC:(s + 1) * BC,
                            ],
                            yo2[:].rearrange("p (c x) -> p c x", c=4),
                        )
    _split_excess_waits(nc)
    return nc


def _get_program():
    if "nc" not in _CACHE:
        _CACHE["nc"] = _build_program()
    return _CACHE["nc"]


def kernel(x: np.ndarray, W: np.ndarray) -> np.ndarray:
    from concourse.bass_utils import run_bass_kernel_spmd

    c1, c2, s2_flat = _build_coeffs(W)
    xT = np.ascontiguousarray(
        np.asarray(x, dtype=np.float32).T.astype(np.float16)
    )  # [L, BATCH] fp16

    nc = _get_program()
    in_maps = []
    for core in range(N_CORES):
        sl = slice(core * BCORE, (core + 1) * BCORE)
        in_maps.append(
            {
                "xT": np.ascontiguousarray(xT[:, sl]),
                "c1": c1,
                "c2": c2,
            }
        )
    _CACHE["in_maps"] = in_maps
    res = run_bass_kernel_spmd(nc, in_maps, list(range(N_CORES)))
    out = np.empty((BATCH, L), dtype=np.float32)
    for core in range(N_CORES):
        deq = res.results[core]["outT"].astype(np.float32) * s2_flat[
            :, None
        ].astype(np.float32)
        out[core * BCORE:(core + 1) * BCORE, :] = deq.T
    return out
